# revision 15
# baseline (speedup 1.0000x reference)
"""Trainium2 Bass kernel for sparse_attention problem nn_CAMD_73229192397362.

All-fp32 (pointwise-rel-faithful to the fp32 reference). Speed comes from
scheduling, not dtype:
  - PE sub-tile concurrency: stacked MLPs on the quadrant diagonal
    (tile_position=(32k,32k)); band S^T matmuls ride the natural quadrant
    rotation of the stacked K^T layout (tile_position=(32k,0)); zo/prefix
    matmuls rotate PE column position by query tile (tile_position=(0,32t))
    writing disjoint PSUM partition pairs.
  - Chunk-major banding: per key chunk ONE S^T matmul + ONE fused mask
    (scalar_tensor_tensor) spanning all query tiles that touch the chunk
    (grouped in 4-tile windows) -> ~75 DVE mask ops instead of 230+.
  - The running prefix state H stays fp32; per-tile H snapshots feed
    per-tile prefix matmuls accumulated in the same PSUM as the band.

Per core (8 cores = 4 modalities x 2 interleaved query half-sets so band
metadata is uniform): Q = MLP(m1) (4096 local queries), K = MLP(m_c),
Z[i] = Q_i . H(w_I) + sum_{j in band, t2[j] <= t1[i]} (Q_i.K_j) V2_j.
"""

import numpy as np

import concourse.bass as bass
from concourse.bacc import Bacc
import concourse.mybir as mybir
from concourse.tile import TileContext
from concourse.bass_utils import run_bass_kernel_spmd

T = 8192
D = 32
TQ = 4096          # queries per core
NT = TQ // 128     # query tiles per core (32)
NCH = T // 128     # key chunks (64)
NG = NT // 4       # zo groups (8)
F32 = mybir.dt.float32
AF = mybir.ActivationFunctionType
OP = mybir.AluOpType


def _stack4(xT):
    """(32, Ttot) -> (128, Ttot//4): 512-col chunk g goes to partition
    block g%4, columns (g//4)*512."""
    d, Ttot = xT.shape
    ng = Ttot // 512
    out = np.zeros((128, Ttot // 4), dtype=xT.dtype)
    for g in range(ng):
        k = g % 4
        out[32 * k:32 * k + 32, (g // 4) * 512:(g // 4) * 512 + 512] = \
            xT[:, g * 512:(g + 1) * 512]
    return out


def _band_meta(t1_all, t2_all):
    """Uniform band metadata. Returns (w, e): per tile the 128-aligned
    band start chunk wc[I]=w[I]//128 and exclusive end chunk e[I], with
    both sequences monotone nondecreasing."""
    w_raw = np.full(NT, T, dtype=np.int64)
    for t1 in t1_all:
        for t2 in t2_all:
            r_min = np.searchsorted(t2, t1[::128], side="right")
            w_raw = np.minimum(w_raw, (r_min // 128) * 128)
    e = np.zeros(NT, dtype=np.int64)
    for t1 in t1_all:
        for t2 in t2_all:
            r_max = np.searchsorted(t2, t1[127::128], side="right")
            e = np.maximum(e, (r_max + 127) // 128)
    wc = w_raw // 128
    e = np.maximum(e, wc + 1)
    e = np.minimum(np.maximum.accumulate(e), NCH)
    wc = np.minimum(wc, e - 1)
    assert np.all(np.diff(wc) >= 0) and np.all(np.diff(e) >= 0)
    for t1 in t1_all:
        for t2 in t2_all:
            r_min = np.searchsorted(t2, t1[::128], side="right")
            r_max = np.searchsorted(t2, t1[127::128], side="right")
            assert np.all(wc * 128 <= r_min) and np.all(r_max <= e * 128)
    return [int(x) for x in wc], [int(x) for x in e]


def _build(wc, e):
    """Build the Bass module (same program for all 8 cores)."""
    import os as _os
    _phase = int(_os.environ.get("BISECT_PHASE", "9"))
    nc = Bacc("TRN2")

    xk = nc.dram_tensor("xk", [128, T // 4], F32, kind="ExternalInput")
    xq = nc.dram_tensor("xq", [128, TQ // 4], F32, kind="ExternalInput")
    wk = nc.dram_tensor("wk", [128, 96], F32, kind="ExternalInput")
    wq = nc.dram_tensor("wq", [128, 96], F32, kind="ExternalInput")
    bk = nc.dram_tensor("bk", [128, 3], F32, kind="ExternalInput")
    bq = nc.dram_tensor("bq", [128, 3], F32, kind="ExternalInput")
    id4 = nc.dram_tensor("id4", [128, 32], F32, kind="ExternalInput")
    t1b = nc.dram_tensor("t1b", [128, TQ], F32, kind="ExternalInput")
    t2p = nc.dram_tensor("t2p", [128, NCH], F32, kind="ExternalInput")
    v2n = nc.dram_tensor("v2n", [128, 2 * NCH], F32, kind="ExternalInput")
    out = nc.dram_tensor("out", [2, TQ], F32, kind="ExternalOutput")

    # host-side chunk geometry --------------------------------------------
    def quad(c):          # PE quadrant of chunk c in the stacked layout
        return (c // 4) % 4

    def ktcol(c):         # kt_s column of chunk c
        return (c // 16) * 512 + (c % 4) * 128

    # group-local band pieces: (c, Ilo, Ihi) with [Ilo,Ihi) inside group g
    pieces = {g: [] for g in range(NG)}
    for c in range(min(wc), max(e)):
        ilo = next((i for i in range(NT) if wc[i] <= c < e[i]), None)
        if ilo is None:
            continue
        ihi = max(i for i in range(NT) if wc[i] <= c < e[i]) + 1
        for i in range(ilo, ihi):
            assert wc[i] <= c < e[i]
        g0, g1 = ilo // 4, (ihi - 1) // 4
        for g in range(g0, g1 + 1):
            lo, hi = max(ilo, 4 * g), min(ihi, 4 * g + 4)
            pieces[g].append((c, lo, hi))
    # quadrant round-robin order within each group
    for g in range(NG):
        by_q = [[p for p in pieces[g] if quad(p[0]) == q] for q in range(4)]
        order = []
        while any(by_q):
            for q in range(4):
                if by_q[q]:
                    order.append(by_q[q].pop(0))
        pieces[g] = order
    max_pieces = max(len(pieces[g]) for g in range(NG))

    with TileContext(nc) as tc:
        with tc.tile_pool(name="cst", bufs=1) as cst, \
             tc.tile_pool(name="big", bufs=1) as big, \
             tc.tile_pool(name="hps", bufs=2, space="PSUM") as hps:

            wk_s = cst.tile([128, 96], F32)
            wq_s = cst.tile([128, 96], F32)
            bk_s = cst.tile([128, 3], F32)
            bq_s = cst.tile([128, 3], F32)
            id4_s = cst.tile([128, 32], F32)
            t1b_s = big.tile([128, TQ], F32, tag="t1b")
            t2p_s = cst.tile([128, NCH], F32)
            v2n_s = cst.tile([128, 2 * NCH], F32)
            xk_s = big.tile([128, T // 4], F32, tag="xk")
            xq_s = big.tile([128, TQ // 4], F32, tag="xq")
            kt_s = big.tile([128, T // 4], F32, tag="kt")   # K^T stacked
            ktf = big.tile([32, T], F32, tag="ktf")          # K^T flat
            knat = big.tile([128, NCH * 32], F32, tag="knat")
            qtf4 = big.tile([32, TQ], F32, tag="qtf4")       # Q^T flat
            zsb4 = cst.tile([128, NG * 128], F32)

            for dst, src in ((wk_s, wk), (wq_s, wq), (bk_s, bk), (bq_s, bq),
                             (id4_s, id4), (t1b_s, t1b), (t2p_s, t2p),
                             (v2n_s, v2n), (xk_s, xk), (xq_s, xq)):
                nc.sync.dma_start(dst[:], src[:])

            # ---------------- MLPs ----------------
            with tc.tile_pool(name="mlp", bufs=4, space="PSUM") as mlp, \
                 tc.tile_pool(name="hbuf", bufs=2) as hbuf:

                def mlp_hidden(x_s, w_s, b_s, ngrp, eng):
                    h_prev = x_s
                    for l in range(2):
                        h_next = hbuf.tile(
                            [128, ngrp * 512], F32, tag=f"h{id(x_s)}")
                        for G in range(ngrp):
                            pt = mlp.tile([128, 512], F32, tag="mlp")
                            for k in range(4):
                                nc.tensor.matmul(
                                    pt[32 * k:32 * k + 32, :],
                                    w_s[32 * k:32 * k + 32, 32 * l:32 * l + 32],
                                    h_prev[32 * k:32 * k + 32,
                                           G * 512:(G + 1) * 512],
                                    start=True, stop=True,
                                    tile_position=(32 * k, 32 * k),
                                )
                            if eng is nc.scalar:
                                eng.activation(
                                    h_next[:, G * 512:(G + 1) * 512], pt[:],
                                    AF.Relu, bias=b_s[:, l:l + 1])
                            else:
                                eng.tensor_scalar(
                                    h_next[:, G * 512:(G + 1) * 512], pt[:],
                                    b_s[:, l:l + 1], 0.0, OP.add, OP.max)
                        h_prev = h_next
                    return h_prev

                h2k = mlp_hidden(xk_s, wk_s, bk_s, 4, nc.vector)
                h2q = mlp_hidden(xq_s, wq_s, bq_s, 2, nc.scalar)

                # K final layer -> stacked kt_s
                for G in range(4):
                    pt = mlp.tile([128, 512], F32, tag="mlp")
                    for k in range(4):
                        nc.tensor.matmul(
                            pt[32 * k:32 * k + 32, :],
                            wk_s[32 * k:32 * k + 32, 64:96],
                            h2k[32 * k:32 * k + 32, G * 512:(G + 1) * 512],
                            start=True, stop=True,
                            tile_position=(32 * k, 32 * k),
                        )
                    if G % 2:
                        nc.scalar.activation(
                            kt_s[:, G * 512:(G + 1) * 512], pt[:],
                            AF.Identity, bias=bk_s[:, 2:3])
                    else:
                        nc.vector.tensor_scalar(
                            kt_s[:, G * 512:(G + 1) * 512], pt[:],
                            bk_s[:, 2:3], None, OP.add)

                # Q final layer -> stacked qts
                qts = hbuf.tile([128, TQ // 4], F32, tag="qts")
                for G in range(2):
                    pt = mlp.tile([128, 512], F32, tag="mlp")
                    for k in range(4):
                        nc.tensor.matmul(
                            pt[32 * k:32 * k + 32, :],
                            wq_s[32 * k:32 * k + 32, 64:96],
                            h2q[32 * k:32 * k + 32, G * 512:(G + 1) * 512],
                            start=True, stop=True,
                            tile_position=(32 * k, 32 * k),
                        )
                    nc.scalar.activation(
                        qts[:, G * 512:(G + 1) * 512], pt[:],
                        AF.Identity, bias=bq_s[:, 2:3])

                # un-stack K and Q to flat (32, x)
                for k in range(4):
                    nc.sync.dma_start(
                        ktf[0:32, :].rearrange(
                            "d (p f) -> d p f", f=512)[:, k::4, :],
                        kt_s[32 * k:32 * k + 32, :])
                for k in range(4):
                    nc.sync.dma_start(
                        qtf4[0:32, :].rearrange(
                            "d (p f) -> d p f", f=512)[:, k::4, :],
                        qts[32 * k:32 * k + 32, :])

                # K natural layout via identity transpose (baseline form)
                for P in range(4):
                    pt = mlp.tile([128, 512], F32, tag="mlp")
                    for j in range(16):
                        c = 16 * P + j
                        nc.tensor.matmul(
                            pt[:, 32 * j:32 * j + 32],
                            ktf[:, 128 * c:128 * c + 128],
                            id4_s[0:32, :],
                            start=True, stop=True,
                        )
                    nc.vector.tensor_copy(
                        knat[:, P * 512:(P + 1) * 512], pt[:])

            # ---------------- attention ----------------
            with tc.tile_pool(name="stp", bufs=4, space="PSUM") as stp, \
                 tc.tile_pool(name="zop", bufs=2, space="PSUM") as zop, \
                 tc.tile_pool(name="smp", bufs=max_pieces + 2) as smp, \
                 tc.tile_pool(name="hsb", bufs=2) as hsbp:

                hsb = hsbp.tile([32, 2], F32, tag="hsb")
                nc.vector.memset(hsb[:], 0)
                delta_done = min(wc)
                if _phase < 1:
                    delta_done = max(e)
                if delta_done > 0 and _phase >= 1:
                    dps = hps.tile([32, 2], F32, tag="dh")
                    for c in range(0, delta_done):
                        nc.tensor.matmul(
                            dps[:], knat[:, 32 * c:32 * c + 32],
                            v2n_s[:, 2 * c:2 * c + 2],
                            start=(c == 0), stop=(c == delta_done - 1))
                    hsb_new = hsbp.tile([32, 2], F32, tag="hsb")
                    nc.vector.tensor_tensor(hsb_new[:], hsb[:], dps[:], OP.add)
                    hsb = hsb_new

                hsb_for = {}
                for g in range(NG):
                    # H snapshots for this group's tiles
                    for t in range(4):
                        I = 4 * g + t
                        if wc[I] > delta_done and _phase >= 1:
                            dps = hps.tile([32, 2], F32, tag="dh")
                            for c in range(delta_done, wc[I]):
                                nc.tensor.matmul(
                                    dps[:], knat[:, 32 * c:32 * c + 32],
                                    v2n_s[:, 2 * c:2 * c + 2],
                                    start=(c == delta_done),
                                    stop=(c == wc[I] - 1))
                            hsb_new = hsbp.tile([32, 2], F32, tag="hsb")
                            nc.vector.tensor_tensor(
                                hsb_new[:], hsb[:], dps[:], OP.add)
                            hsb = hsb_new
                            delta_done = wc[I]
                        hsb_for[I] = hsb

                    # chunk-major band S^T + fused mask over tile spans
                    smt_of = {}
                    for (c, lo, hi) in (pieces[g] if _phase >= 3 else []):
                        wd = 128 * (hi - lo)
                        k = quad(c)
                        stb = stp.tile([128, 512], F32, tag="st")
                        nc.tensor.matmul(
                            stb[:, 0:wd],
                            ktf[:, 128 * c:128 * c + 128],
                            qtf4[0:32, 128 * lo:128 * hi],
                            start=True, stop=True,
                        )
                        smt = smp.tile([128, 512], F32, tag="smt")
                        nc.vector.scalar_tensor_tensor(
                            smt[:, 0:wd],
                            t1b_s[:, 128 * lo:128 * hi],
                            t2p_s[:, c:c + 1],
                            stb[:, 0:wd],
                            OP.is_ge, OP.mult)
                        smt_of[c] = (smt, lo)

                    # per-tile prefix + band zo into rotated PSUM columns
                    zo4 = zop.tile([128, 128], F32, tag="zo")
                    for t in (range(4) if _phase >= 1 else []):
                        I = 4 * g + t
                        nc.tensor.matmul(
                            zo4[32 * t:32 * t + 2, :], hsb_for[I][:],
                            qtf4[0:32, 128 * I:128 * I + 128],
                            start=True, stop=(_phase < 4),
                            tile_position=(0, 32 * t))
                    for t in range(4):
                        I = 4 * g + t
                        for c in (range(wc[I], e[I]) if _phase >= 4 else []):
                            smt, lo = smt_of[c]
                            off = 128 * (I - lo)
                            nc.tensor.matmul(
                                zo4[32 * t:32 * t + 2, :],
                                v2n_s[:, 2 * c:2 * c + 2],
                                smt[:, off:off + 128],
                                start=False, stop=(c == e[I] - 1),
                                tile_position=(0, 32 * t))
                    for t in (range(4) if _phase >= 1 else []):
                        nc.scalar.activation(
                            zsb4[32 * t:32 * t + 2, 128 * g:128 * g + 128],
                            zo4[32 * t:32 * t + 2, :], AF.Copy)

                # stitch (2, TQ): query 128*(4g+t)+c  <-  zsb4[32t+v, 128g+c]
                if _phase >= 2:
                    for t in range(4):
                        nc.sync.dma_start(
                            out[:, :].rearrange(
                                "p (g f c) -> p g f c", f=4, c=128)[:, :, t:t + 1, :],
                            zsb4[32 * t:32 * t + 2, :].rearrange(
                                "p (g c) -> p g c", c=128))
                else:
                    nc.sync.dma_start(out[:], qtf4[0:2, :])
    nc.finalize()
    return nc


_CACHE = {}
LAST_RESULTS = None


def kernel(m1, m2, m3, m4, Wq, bq, Wk, bk):
    mods = [np.asarray(m)[0, 0].astype(np.float32) for m in (m1, m2, m3, m4)]
    Wq, bq, Wk, bk = (np.asarray(a, dtype=np.float32) for a in (Wq, bq, Wk, bk))
    t2s = [m[:, -1].copy() for m in mods]
    t1g = mods[0][:, -1].copy()

    # core c: modality c//2, half h=c%2 takes global query tiles 2I+h
    def qsel(h):
        idx = np.arange(TQ)
        gt = 2 * (idx // 128) + h
        return gt * 128 + (idx % 128)

    sels = [qsel(0), qsel(1)]
    t1_locals = [t1g[s] for s in sels]
    wc, e = _band_meta(t1_locals, t2s)

    key = (tuple(wc), tuple(e))
    if key not in _CACHE:
        _CACHE[key] = _build(wc, e)
    nc = _CACHE[key]

    wq_in = np.tile(np.concatenate([Wq[l] for l in range(3)], axis=1), (4, 1))
    wk_in = np.tile(np.concatenate([Wk[l] for l in range(3)], axis=1), (4, 1))
    bq_in = np.tile(bq.T, (4, 1)).astype(np.float32)
    bk_in = np.tile(bk.T, (4, 1)).astype(np.float32)
    id4_in = np.tile(np.eye(32, dtype=np.float32), (4, 1))

    in_maps = []
    for c in range(8):
        mod, h = c // 2, c % 2
        x = mods[mod]
        t2 = t2s[mod]
        xk_in = _stack4(np.ascontiguousarray(x.T))
        xq_l = mods[0][sels[h]]
        xq_in = _stack4(np.ascontiguousarray(xq_l.T))
        t1b_in = np.ascontiguousarray(
            np.broadcast_to(t1_locals[h], (128, TQ))).astype(np.float32)
        t2p_in = np.ascontiguousarray(t2.reshape(NCH, 128).T)
        v2n_in = np.ascontiguousarray(
            x[:, :2].reshape(NCH, 128, 2).transpose(1, 0, 2)
            .reshape(128, 2 * NCH))
        in_maps.append({
            "xk": xk_in, "xq": xq_in, "wk": wk_in, "wq": wq_in,
            "bk": bk_in, "bq": bq_in, "id4": id4_in, "t1b": t1b_in,
            "t2p": t2p_in, "v2n": v2n_in,
        })

    import os as _os
    trace = bool(_os.environ.get("KERNEL_TRACE"))
    res = run_bass_kernel_spmd(nc, in_maps, core_ids=list(range(8)),
                               trace=trace)
    global LAST_RESULTS
    LAST_RESULTS = res

    y = np.zeros((T, 2), dtype=np.float32)
    for c in range(8):
        mod, h = c // 2, c % 2
        zt = res.results[c]["out"]          # (2, TQ) local order
        y[sels[h]] += zt.T
    return y[None, :, :]


# revision 16
# speedup vs baseline: 1.0186x; 1.0186x over previous
"""Trainium2 Bass kernel for sparse_attention problem nn_CAMD_73229192397362.

All-fp32 (pointwise-rel-faithful to the fp32 reference). Speed comes from
scheduling, not dtype:
  - PE sub-tile concurrency: stacked MLPs on the quadrant diagonal
    (tile_position=(32k,32k)); band S^T matmuls ride the natural quadrant
    rotation of the stacked K^T layout (tile_position=(32k,0)); zo/prefix
    matmuls rotate PE column position by query tile (tile_position=(0,32t))
    writing disjoint PSUM partition pairs.
  - Chunk-major banding: per key chunk ONE S^T matmul + ONE fused mask
    (scalar_tensor_tensor) spanning all query tiles that touch the chunk
    (grouped in 4-tile windows) -> ~75 DVE mask ops instead of 230+.
  - The running prefix state H stays fp32; per-tile H snapshots feed
    per-tile prefix matmuls accumulated in the same PSUM as the band.

Per core (8 cores = 4 modalities x 2 interleaved query half-sets so band
metadata is uniform): Q = MLP(m1) (4096 local queries), K = MLP(m_c),
Z[i] = Q_i . H(w_I) + sum_{j in band, t2[j] <= t1[i]} (Q_i.K_j) V2_j.
"""

import numpy as np

import concourse.bass as bass
from concourse.bacc import Bacc
import concourse.mybir as mybir
from concourse.tile import TileContext
from concourse.bass_utils import run_bass_kernel_spmd

T = 8192
D = 32
TQ = 4096          # queries per core
NT = TQ // 128     # query tiles per core (32)
NCH = T // 128     # key chunks (64)
NG = NT // 4       # zo groups (8)
F32 = mybir.dt.float32
AF = mybir.ActivationFunctionType
OP = mybir.AluOpType


def _stack4(xT):
    """(32, Ttot) -> (128, Ttot//4): 512-col chunk g goes to partition
    block g%4, columns (g//4)*512."""
    d, Ttot = xT.shape
    ng = Ttot // 512
    out = np.zeros((128, Ttot // 4), dtype=xT.dtype)
    for g in range(ng):
        k = g % 4
        out[32 * k:32 * k + 32, (g // 4) * 512:(g // 4) * 512 + 512] = \
            xT[:, g * 512:(g + 1) * 512]
    return out


def _band_meta(t1_all, t2_all):
    """Uniform band metadata. Returns (w, e): per tile the 128-aligned
    band start chunk wc[I]=w[I]//128 and exclusive end chunk e[I], with
    both sequences monotone nondecreasing."""
    w_raw = np.full(NT, T, dtype=np.int64)
    for t1 in t1_all:
        for t2 in t2_all:
            r_min = np.searchsorted(t2, t1[::128], side="right")
            w_raw = np.minimum(w_raw, (r_min // 128) * 128)
    e = np.zeros(NT, dtype=np.int64)
    for t1 in t1_all:
        for t2 in t2_all:
            r_max = np.searchsorted(t2, t1[127::128], side="right")
            e = np.maximum(e, (r_max + 127) // 128)
    wc = w_raw // 128
    e = np.maximum(e, wc + 1)
    e = np.minimum(np.maximum.accumulate(e), NCH)
    wc = np.minimum(wc, e - 1)
    assert np.all(np.diff(wc) >= 0) and np.all(np.diff(e) >= 0)
    for t1 in t1_all:
        for t2 in t2_all:
            r_min = np.searchsorted(t2, t1[::128], side="right")
            r_max = np.searchsorted(t2, t1[127::128], side="right")
            assert np.all(wc * 128 <= r_min) and np.all(r_max <= e * 128)
    return [int(x) for x in wc], [int(x) for x in e]


def _build(wc, e):
    """Build the Bass module (same program for all 8 cores)."""
    import os as _os
    _phase = int(_os.environ.get("BISECT_PHASE", "9"))
    nc = Bacc("TRN2")

    xk = nc.dram_tensor("xk", [128, T // 4], F32, kind="ExternalInput")
    xq = nc.dram_tensor("xq", [128, TQ // 4], F32, kind="ExternalInput")
    wk = nc.dram_tensor("wk", [128, 96], F32, kind="ExternalInput")
    wq = nc.dram_tensor("wq", [128, 96], F32, kind="ExternalInput")
    bk = nc.dram_tensor("bk", [128, 3], F32, kind="ExternalInput")
    bq = nc.dram_tensor("bq", [128, 3], F32, kind="ExternalInput")
    id4 = nc.dram_tensor("id4", [128, 32], F32, kind="ExternalInput")
    t1b = nc.dram_tensor("t1b", [128, TQ], F32, kind="ExternalInput")
    t2p = nc.dram_tensor("t2p", [128, NCH], F32, kind="ExternalInput")
    v2n = nc.dram_tensor("v2n", [128, 2 * NCH], F32, kind="ExternalInput")
    out = nc.dram_tensor("out", [2, TQ], F32, kind="ExternalOutput")

    # host-side chunk geometry --------------------------------------------
    def quad(c):          # PE quadrant of chunk c in the stacked layout
        return (c // 4) % 4

    def ktcol(c):         # kt_s column of chunk c
        return (c // 16) * 512 + (c % 4) * 128

    # group-local band pieces: (c, Ilo, Ihi) with [Ilo,Ihi) inside group g
    pieces = {g: [] for g in range(NG)}
    for c in range(min(wc), max(e)):
        ilo = next((i for i in range(NT) if wc[i] <= c < e[i]), None)
        if ilo is None:
            continue
        ihi = max(i for i in range(NT) if wc[i] <= c < e[i]) + 1
        for i in range(ilo, ihi):
            assert wc[i] <= c < e[i]
        g0, g1 = ilo // 4, (ihi - 1) // 4
        for g in range(g0, g1 + 1):
            lo, hi = max(ilo, 4 * g), min(ihi, 4 * g + 4)
            pieces[g].append((c, lo, hi))
    # quadrant round-robin order within each group
    for g in range(NG):
        by_q = [[p for p in pieces[g] if quad(p[0]) == q] for q in range(4)]
        order = []
        while any(by_q):
            for q in range(4):
                if by_q[q]:
                    order.append(by_q[q].pop(0))
        pieces[g] = order
    max_pieces = max(len(pieces[g]) for g in range(NG))

    with TileContext(nc) as tc:
        with tc.tile_pool(name="cst", bufs=1) as cst, \
             tc.tile_pool(name="big", bufs=1) as big, \
             tc.tile_pool(name="hps", bufs=2, space="PSUM") as hps:

            wk_s = cst.tile([128, 96], F32)
            wq_s = cst.tile([128, 96], F32)
            bk_s = cst.tile([128, 3], F32)
            bq_s = cst.tile([128, 3], F32)
            id4_s = cst.tile([128, 32], F32)
            t1b_s = big.tile([128, TQ], F32, tag="t1b")
            t2p_s = cst.tile([128, NCH], F32)
            v2n_s = cst.tile([128, 2 * NCH], F32)
            xk_s = big.tile([128, T // 4], F32, tag="xk")
            xq_s = big.tile([128, TQ // 4], F32, tag="xq")
            kt_s = big.tile([128, T // 4], F32, tag="kt")   # K^T stacked
            ktf = big.tile([32, T], F32, tag="ktf")          # K^T flat
            knat = big.tile([128, NCH * 32], F32, tag="knat")
            qtf4 = big.tile([32, TQ], F32, tag="qtf4")       # Q^T flat
            zsb4 = cst.tile([128, NG * 128], F32)

            for dst, src in ((wk_s, wk), (wq_s, wq), (bk_s, bk), (bq_s, bq),
                             (id4_s, id4), (t1b_s, t1b), (t2p_s, t2p),
                             (v2n_s, v2n), (xk_s, xk), (xq_s, xq)):
                nc.sync.dma_start(dst[:], src[:])

            # ---------------- MLPs ----------------
            with tc.tile_pool(name="mlp", bufs=3, space="PSUM") as mlp, \
                 tc.tile_pool(name="hbuf", bufs=2) as hbuf:

                def mlp_hidden(x_s, w_s, b_s, ngrp, eng):
                    h_prev = x_s
                    for l in range(2):
                        h_next = hbuf.tile(
                            [128, ngrp * 512], F32, tag=f"h{id(x_s)}")
                        for G in range(ngrp):
                            pt = mlp.tile([128, 512], F32, tag="mlp")
                            for k in range(4):
                                nc.tensor.matmul(
                                    pt[32 * k:32 * k + 32, :],
                                    w_s[32 * k:32 * k + 32, 32 * l:32 * l + 32],
                                    h_prev[32 * k:32 * k + 32,
                                           G * 512:(G + 1) * 512],
                                    start=True, stop=True,
                                    tile_position=(32 * k, 32 * k),
                                )
                            if eng is nc.scalar:
                                eng.activation(
                                    h_next[:, G * 512:(G + 1) * 512], pt[:],
                                    AF.Relu, bias=b_s[:, l:l + 1])
                            else:
                                eng.tensor_scalar(
                                    h_next[:, G * 512:(G + 1) * 512], pt[:],
                                    b_s[:, l:l + 1], 0.0, OP.add, OP.max)
                        h_prev = h_next
                    return h_prev

                h2k = mlp_hidden(xk_s, wk_s, bk_s, 4, nc.vector)
                h2q = mlp_hidden(xq_s, wq_s, bq_s, 2, nc.scalar)

                # K final layer -> stacked kt_s
                for G in range(4):
                    pt = mlp.tile([128, 512], F32, tag="mlp")
                    for k in range(4):
                        nc.tensor.matmul(
                            pt[32 * k:32 * k + 32, :],
                            wk_s[32 * k:32 * k + 32, 64:96],
                            h2k[32 * k:32 * k + 32, G * 512:(G + 1) * 512],
                            start=True, stop=True,
                            tile_position=(32 * k, 32 * k),
                        )
                    if G % 2:
                        nc.scalar.activation(
                            kt_s[:, G * 512:(G + 1) * 512], pt[:],
                            AF.Identity, bias=bk_s[:, 2:3])
                    else:
                        nc.vector.tensor_scalar(
                            kt_s[:, G * 512:(G + 1) * 512], pt[:],
                            bk_s[:, 2:3], None, OP.add)

                # Q final layer -> stacked qts
                qts = hbuf.tile([128, TQ // 4], F32, tag="qts")
                for G in range(2):
                    pt = mlp.tile([128, 512], F32, tag="mlp")
                    for k in range(4):
                        nc.tensor.matmul(
                            pt[32 * k:32 * k + 32, :],
                            wq_s[32 * k:32 * k + 32, 64:96],
                            h2q[32 * k:32 * k + 32, G * 512:(G + 1) * 512],
                            start=True, stop=True,
                            tile_position=(32 * k, 32 * k),
                        )
                    nc.scalar.activation(
                        qts[:, G * 512:(G + 1) * 512], pt[:],
                        AF.Identity, bias=bq_s[:, 2:3])

                # un-stack K and Q to flat (32, x)
                for k in range(4):
                    nc.sync.dma_start(
                        ktf[0:32, :].rearrange(
                            "d (p f) -> d p f", f=512)[:, k::4, :],
                        kt_s[32 * k:32 * k + 32, :])
                for k in range(4):
                    nc.sync.dma_start(
                        qtf4[0:32, :].rearrange(
                            "d (p f) -> d p f", f=512)[:, k::4, :],
                        qts[32 * k:32 * k + 32, :])

                # K natural layout via identity transpose (baseline form)
                for P in range(4):
                    pt = mlp.tile([128, 512], F32, tag="mlp")
                    for j in range(16):
                        c = 16 * P + j
                        nc.tensor.matmul(
                            pt[:, 32 * j:32 * j + 32],
                            ktf[:, 128 * c:128 * c + 128],
                            id4_s[0:32, :],
                            start=True, stop=True,
                        )
                    nc.vector.tensor_copy(
                        knat[:, P * 512:(P + 1) * 512], pt[:])

            # ---------------- attention ----------------
            with tc.tile_pool(name="stp", bufs=4, space="PSUM") as stp, \
                 tc.tile_pool(name="zop", bufs=2, space="PSUM") as zop, \
                 tc.tile_pool(name="smp", bufs=max_pieces + 2) as smp, \
                 tc.tile_pool(name="hsb", bufs=2) as hsbp:

                hsb = hsbp.tile([32, 2], F32, tag="hsb")
                nc.vector.memset(hsb[:], 0)
                delta_done = min(wc)
                if _phase < 1:
                    delta_done = max(e)
                if delta_done > 0 and _phase >= 1:
                    dps = hps.tile([32, 2], F32, tag="dh")
                    for c in range(0, delta_done):
                        nc.tensor.matmul(
                            dps[:], knat[:, 32 * c:32 * c + 32],
                            v2n_s[:, 2 * c:2 * c + 2],
                            start=(c == 0), stop=(c == delta_done - 1))
                    hsb_new = hsbp.tile([32, 2], F32, tag="hsb")
                    nc.vector.tensor_tensor(hsb_new[:], hsb[:], dps[:], OP.add)
                    hsb = hsb_new

                hsb_for = {}
                for g in range(NG):
                    # H snapshots for this group's tiles
                    for t in range(4):
                        I = 4 * g + t
                        if wc[I] > delta_done and _phase >= 1:
                            dps = hps.tile([32, 2], F32, tag="dh")
                            for c in range(delta_done, wc[I]):
                                nc.tensor.matmul(
                                    dps[:], knat[:, 32 * c:32 * c + 32],
                                    v2n_s[:, 2 * c:2 * c + 2],
                                    start=(c == delta_done),
                                    stop=(c == wc[I] - 1))
                            hsb_new = hsbp.tile([32, 2], F32, tag="hsb")
                            nc.vector.tensor_tensor(
                                hsb_new[:], hsb[:], dps[:], OP.add)
                            hsb = hsb_new
                            delta_done = wc[I]
                        hsb_for[I] = hsb

                    # chunk-major band S^T + fused mask over tile spans
                    smt_of = {}
                    for (c, lo, hi) in (pieces[g] if _phase >= 3 else []):
                        wd = 128 * (hi - lo)
                        k = quad(c)
                        stb = stp.tile([128, 512], F32, tag="st")
                        nc.tensor.matmul(
                            stb[:, 0:wd],
                            ktf[:, 128 * c:128 * c + 128],
                            qtf4[0:32, 128 * lo:128 * hi],
                            start=True, stop=True,
                        )
                        smt = smp.tile([128, 512], F32, tag="smt")
                        nc.vector.scalar_tensor_tensor(
                            smt[:, 0:wd],
                            t1b_s[:, 128 * lo:128 * hi],
                            t2p_s[:, c:c + 1],
                            stb[:, 0:wd],
                            OP.is_ge, OP.mult)
                        smt_of[c] = (smt, lo)

                    # per-tile prefix + band zo into rotated PSUM columns
                    zo4 = zop.tile([128, 128], F32, tag="zo")
                    for t in (range(4) if _phase >= 1 else []):
                        I = 4 * g + t
                        nc.tensor.matmul(
                            zo4[32 * t:32 * t + 2, :], hsb_for[I][:],
                            qtf4[0:32, 128 * I:128 * I + 128],
                            start=True, stop=(_phase < 4),
                            tile_position=(0, 32 * t))
                    for t in range(4):
                        I = 4 * g + t
                        for c in (range(wc[I], e[I]) if _phase >= 4 else []):
                            smt, lo = smt_of[c]
                            off = 128 * (I - lo)
                            nc.tensor.matmul(
                                zo4[32 * t:32 * t + 2, :],
                                v2n_s[:, 2 * c:2 * c + 2],
                                smt[:, off:off + 128],
                                start=False, stop=(c == e[I] - 1),
                                tile_position=(0, 32 * t))
                    for t in (range(4) if _phase >= 1 else []):
                        nc.scalar.activation(
                            zsb4[32 * t:32 * t + 2, 128 * g:128 * g + 128],
                            zo4[32 * t:32 * t + 2, :], AF.Copy)

                # stitch (2, TQ): query 128*(4g+t)+c  <-  zsb4[32t+v, 128g+c]
                if _phase >= 2:
                    for t in range(4):
                        nc.sync.dma_start(
                            out[:, :].rearrange(
                                "p (g f c) -> p g f c", f=4, c=128)[:, :, t:t + 1, :],
                            zsb4[32 * t:32 * t + 2, :].rearrange(
                                "p (g c) -> p g c", c=128))
                else:
                    nc.sync.dma_start(out[:], qtf4[0:2, :])
    nc.finalize()
    return nc


_CACHE = {}
LAST_RESULTS = None


def kernel(m1, m2, m3, m4, Wq, bq, Wk, bk):
    mods = [np.asarray(m)[0, 0].astype(np.float32) for m in (m1, m2, m3, m4)]
    Wq, bq, Wk, bk = (np.asarray(a, dtype=np.float32) for a in (Wq, bq, Wk, bk))
    t2s = [m[:, -1].copy() for m in mods]
    t1g = mods[0][:, -1].copy()

    # core c: modality c//2, half h=c%2 takes global query tiles 2I+h
    def qsel(h):
        idx = np.arange(TQ)
        gt = 2 * (idx // 128) + h
        return gt * 128 + (idx % 128)

    sels = [qsel(0), qsel(1)]
    t1_locals = [t1g[s] for s in sels]
    wc, e = _band_meta(t1_locals, t2s)

    key = (tuple(wc), tuple(e))
    if key not in _CACHE:
        _CACHE[key] = _build(wc, e)
    nc = _CACHE[key]

    wq_in = np.tile(np.concatenate([Wq[l] for l in range(3)], axis=1), (4, 1))
    wk_in = np.tile(np.concatenate([Wk[l] for l in range(3)], axis=1), (4, 1))
    bq_in = np.tile(bq.T, (4, 1)).astype(np.float32)
    bk_in = np.tile(bk.T, (4, 1)).astype(np.float32)
    id4_in = np.tile(np.eye(32, dtype=np.float32), (4, 1))

    in_maps = []
    for c in range(8):
        mod, h = c // 2, c % 2
        x = mods[mod]
        t2 = t2s[mod]
        xk_in = _stack4(np.ascontiguousarray(x.T))
        xq_l = mods[0][sels[h]]
        xq_in = _stack4(np.ascontiguousarray(xq_l.T))
        t1b_in = np.ascontiguousarray(
            np.broadcast_to(t1_locals[h], (128, TQ))).astype(np.float32)
        t2p_in = np.ascontiguousarray(t2.reshape(NCH, 128).T)
        v2n_in = np.ascontiguousarray(
            x[:, :2].reshape(NCH, 128, 2).transpose(1, 0, 2)
            .reshape(128, 2 * NCH))
        in_maps.append({
            "xk": xk_in, "xq": xq_in, "wk": wk_in, "wq": wq_in,
            "bk": bk_in, "bq": bq_in, "id4": id4_in, "t1b": t1b_in,
            "t2p": t2p_in, "v2n": v2n_in,
        })

    import os as _os
    trace = bool(_os.environ.get("KERNEL_TRACE"))
    res = run_bass_kernel_spmd(nc, in_maps, core_ids=list(range(8)),
                               trace=trace)
    global LAST_RESULTS
    LAST_RESULTS = res

    y = np.zeros((T, 2), dtype=np.float32)
    for c in range(8):
        mod, h = c // 2, c % 2
        zt = res.results[c]["out"]          # (2, TQ) local order
        y[sels[h]] += zt.T
    return y[None, :, :]
